# revision 38
# baseline (speedup 1.0000x reference)
"""Multi-head self-attention (B=2, S=4096, D=512, H=8, Dk=64) on 8 TRN2 cores.

Sharding: data-parallel over batch x head-parallel. Core c handles batch
c//4 and head pair (2*(c%4), 2*(c%4)+1). Each core computes Q/K/V
projections for its 128 model dims, full attention for its two heads, and
a partial output projection against its 128 rows of Wo. The host sums the
four partial outputs per batch and adds bo.

x arrives host-transposed as xT [512, S] bf16, streamed in 512-token
chunks; chunk-0 projections run up front and the remaining chunks'
K/Q/V projections are staggered into query-block 0's key loop so
attention (and the exp stream, the critical resource) starts as soon as
chunk 0 lands instead of after all projections.

The softmax exp is split across TWO engines: the Scalar (ACT) engine
computes exact exp for most key blocks, and the Vector (DVE) engine
computes a Schraudolph-style exp2 approximation (one tensor_scalar op:
round(s*A + B) -> int16, bit-cast as bf16) for DVE_QB blocks per query
block. ACT throughput is (N+352)/1.2 ns per [128, N] block and exp
exists only on ACT, so offloading ~40% of blocks to the otherwise-idle
DVE removes the single-engine exp floor (~294 us). The approximation
carries +-3% per-weight error; softmax renormalization cancels the mean
and the verified end-to-end rel err is ~1.3e-2 (gate 2e-2).

On-core layout (bf16 operands, fp32 psum accumulation):
  xT   [d, s]   bf16, host-transposed        (rhs for Q/K, lhsT for V)
  QT/KT [128, S] bf16, head0 in partitions 0:64, head1 in 64:128
  V    [s, 128] bf16, head0 in cols 0:64, head1 in 64:128 (lhsT for ctx)
  scoresT[k, q] fp32 psum from row-paired bf16 matmuls (K=64/head)
  attnT = exp(scoresT/8) bf16, per [128, 1024] block on ACT or DVE
  ctxT [d, q] fp32 psum, col-paired over k blocks; denominators from
  ones-vector matmuls into psum rows 0/32; reciprocal_approx_fast reads
  them straight from psum; normalization via fp32 PE broadcast.
"""

import numpy as np
import ml_dtypes
from contextlib import ExitStack

import concourse.bass as bass
import concourse.tile as tile
from concourse import bacc, mybir
from concourse.bass_utils import run_bass_kernel_spmd
from concourse.tile_rust import add_dep_helper

F32 = mybir.dt.float32
F16 = mybir.dt.float16
BF16 = mybir.dt.bfloat16
I16 = mybir.dt.int16
EXP = mybir.ActivationFunctionType.Exp

D_MODEL = 512
N_HEADS = 8
D_K = 64
N_CORES = 8
DL = 128          # local model dims per core (2 heads)
Q_BLK = 512       # query block (free dim of scores matmuls)
SCALE = 1.0 / np.sqrt(D_K).item()

# Schraudolph exp2 on DVE: exp(s/8) ~ bf16-bitcast(int16(round(s*A + B)))
LOG2E = 1.4426950408889634
SCH_A = 128.0 * LOG2E * SCALE
SCH_B = 128.0 * (127.0 - 0.0434)

# Most blocks split their exp half ACT (exact) / half DVE (Schraudolph),
# alternating per block so every (query, head) sees the DVE path on half
# its key blocks. Halving the free dim halves the exp latency that gates
# reuse of the two scores-psum ring slots. A few blocks per qb run fully
# on ACT: they cut the per-query approximated fraction to 14/32 and give
# the DVE catch-up room for its tail work.
ACT_KBS = (1, 9, 17, 25)


def build_kernel(ctx, tc, S, use_mask, use_bq, use_bk, use_bv, d):
    nc = tc.nc
    SB = S // 128    # s blocks of 128
    QB = S // Q_BLK  # query blocks of 512
    KB = S // 128    # key blocks of 128
    CHUNK = 512
    NCH = S // CHUNK

    sp = ctx.enter_context(tc.tile_pool(name="sp", bufs=1))
    psum = ctx.enter_context(tc.tile_pool(name="psum", bufs=1, space="PSUM"))
    # psum budget (8 banks): scores 2x[128,1024]=4, ctx 2x[128,512]=2,
    # den 2x[<=1 bank]=2. All other matmul outputs share the ctx/den tags.

    # ---- constants ----
    ones_f = sp.tile([128, 1], F32, tag="ones_f")
    nc.vector.memset(ones_f, 1.0)
    ones_col = sp.tile([128, 1], BF16, tag="ones_col")  # lhsT of denominator mms
    nc.vector.tensor_copy(ones_col, ones_f)
    # host-prepared selector: one matmul sel4.T @ pd_sb both SUMS the
    # even/odd denominator halves (rows {0,64} per head0, {32,96} head1)
    # and BROADCASTS the result across the 128 output partitions
    # (cols 0:64 pick head0, 64:128 head1)
    sel4 = sp.tile([97, 128], F16, tag="sel4")
    nc.sync.dma_start(sel4, d["sel4"].ap())

    # ---- DMA in: weights first (small; needed by every projection), then
    # x (host-transposed bf16) in 512-token chunks so chunk-0 projections
    # and the first exp start as early as possible. ----
    wq_sb = sp.tile([128, 4, 128], BF16, tag="wq")
    nc.sync.dma_start(wq_sb, d["wq"].ap().rearrange("(t p) d -> p t d", p=128))
    wk_sb = sp.tile([128, 4, 128], BF16, tag="wk")
    nc.sync.dma_start(wk_sb, d["wk"].ap().rearrange("(t p) d -> p t d", p=128))
    wv_sb = sp.tile([128, 4, 128], BF16, tag="wv")
    nc.sync.dma_start(wv_sb, d["wv"].ap().rearrange("(t p) d -> p t d", p=128))
    wo_sb = sp.tile([128, 512], BF16, tag="wo")
    nc.sync.dma_start(wo_sb, d["wo"].ap())
    if use_bq:
        bq_sb = sp.tile([128, 1], F32, tag="bq")
        nc.sync.dma_start(bq_sb, d["bq"].ap()[:, None])
    if use_bk:
        bk_sb = sp.tile([128, 1], F32, tag="bk")
        nc.sync.dma_start(bk_sb, d["bk"].ap()[:, None])
    if use_bv:
        bv_sb = sp.tile([128, 1], F32, tag="bv")
        nc.sync.dma_start(bv_sb, d["bv"].ap()[:, None])
    idn = sp.tile([128, 128], BF16, tag="idn")
    nc.sync.dma_start(idn, d["idn"].ap())
    if use_mask:
        mb_sb = sp.tile([128, KB], F32, tag="mb")
        nc.sync.dma_start(mb_sb, d["mb"].ap())

    xt = sp.tile([128, 4, S], BF16, tag="xt")
    xsrc = d["xt"].ap().rearrange("(t p) s -> p t s", p=128)
    for c in range(NCH):
        cs = slice(c * CHUNK, (c + 1) * CHUNK)
        nc.sync.dma_start(xt[:, :, cs], xsrc[:, :, cs])

    # ---- PE warm-up: the HAM clock gate needs ~3.4us of sustained matmul
    # activity to lift the PE from 1.2 to 2.4 GHz; run throwaway matmuls
    # while x streams in so the projections start at full clock. ----
    scratch = sp.tile([128, 512], BF16, tag="scratch")
    nc.vector.memset(scratch, 0.0)
    for _ in range(12):
        pw = psum.tile([33, 512], F32, tag="den", bufs=2, name="pw")
        nc.tensor.matmul(pw[0:1, :], scratch[:, 0:1], scratch)

    # ---- projections ----
    # V is projected TRANSPOSED (vt [dk, s], weight-stationary N=512 slots
    # like Q/K -- ~3x cheaper than s-stationary N=128 chains) and each
    # 128-token block is then PE-transposed into ctx-matmul orientation.
    qt = sp.tile([128, S], BF16, tag="qt")
    kt = sp.tile([128, S], BF16, tag="kt")
    vt = sp.tile([128, S], BF16, tag="vt")
    v_all = sp.tile([128, SB, 128], BF16, tag="v")

    def proj_qk(dst, w_sb, b_sb, c):
        # one 512-token sub-chunk of the Q/K/Vt projection
        pp = psum.tile([128, 512], F32, tag="ctx", bufs=2, name="pp")
        for t in range(4):
            nc.tensor.matmul(pp, w_sb[:, t, :], xt[:, t, c * 512:(c + 1) * 512],
                             start=(t == 0), stop=(t == 3))
        out = dst[:, c * 512:(c + 1) * 512]
        if b_sb is not None:
            nc.vector.tensor_scalar_add(out, pp, b_sb[:, 0:1])
        else:
            nc.vector.tensor_copy(out, pp)

    def proj_v(sb):
        pv = psum.tile([128, 128], BF16, tag="den", bufs=2, name="pv")
        nc.tensor.transpose(pv, vt[:, sb * 128:(sb + 1) * 128], idn)
        nc.vector.tensor_copy(v_all[:, sb, :], pv)

    bkk = bk_sb if use_bk else None
    bqq = bq_sb if use_bq else None
    bvv = bv_sb if use_bv else None

    # chunk 0 up front: everything query-block 0 needs to start
    proj_qk(kt, wk_sb, bkk, 0)
    proj_qk(qt, wq_sb, bqq, 0)
    proj_qk(vt, wv_sb, bvv, 0)
    for sb in range(4):
        proj_v(sb)

    # chunks 1..NCH-1 staggered into qb0's units. Chunk c's K/V blocks are
    # first consumed at global block 4c (scores emitted at unit 2c-2), so
    # emit them in the two units before that, DMA-gated by Tile deps.
    pending = {}

    def make_chunk_stage(c, part):
        if part == 0:
            return lambda: (proj_qk(kt, wk_sb, bkk, c),
                            proj_qk(vt, wv_sb, bvv, c),
                            proj_v(4 * c), proj_v(4 * c + 1))
        return lambda: (proj_v(4 * c + 2), proj_v(4 * c + 3),
                        proj_qk(qt, wq_sb, bqq, c))

    # chunks 1-2 fit in the prologue's DMA shadow (PE otherwise idles
    # while later chunks stream in); 3+ stagger into qb0's units
    for c in range(1, NCH):
        if c <= 2:
            make_chunk_stage(c, 0)()
            make_chunk_stage(c, 1)()
        else:
            u = 2 * (c - 3)
            pending.setdefault(u, []).append(make_chunk_stage(c, 0))
            pending.setdefault(u + 1, []).append(make_chunk_stage(c, 1))

    # ---- attention: one flat loop over 2-block units ----
    # Per unit (blocks g0=2u, g1=2u+1): ctx pair for g0, scores pair for
    # g0+4, ctx pair for g1, scores pair for g1+4, then ONE fused 4-way
    # denominator slot (M=1 matmuls on col groups 0/32/64/96 of pd).
    # The 4-block scores lookahead keeps exp latency off the ctx critical
    # path; even blocks always use ACT so the even psum-ring slot is
    # ready when scores(g+4) needs it.
    NBLK = QB * KB
    ctxn = sp.tile([128, S], BF16, tag="ctxn")

    def scores_block(g):
        qb, kb = divmod(g, KB)
        qs = slice(qb * Q_BLK, (qb + 1) * Q_BLK)
        ks = slice(kb * 128, (kb + 1) * 128)
        ps = psum.tile([128, 1024], F32, tag="scores", bufs=2, name="ps")
        nc.tensor.matmul(ps[:, 0:512], kt[0:64, ks], qt[0:64, qs])
        nc.tensor.matmul(ps[:, 512:1024], kt[64:128, ks], qt[64:128, qs])
        attn = sp.tile([128, 1024], BF16, tag="attn", bufs=6, name="attn")
        if not use_mask and kb not in ACT_KBS:
            lo, hi = (slice(0, 512), slice(512, 1024))
            if g % 2:
                lo, hi = hi, lo
            nc.scalar.activation(attn[:, lo], ps[:, lo], EXP, scale=SCALE,
                                 bias=0.0)
            nc.vector.tensor_scalar(
                attn.bitcast(I16)[:, hi], ps[:, hi], SCH_A, SCH_B,
                mybir.AluOpType.mult, mybir.AluOpType.add)
        else:
            nc.scalar.activation(
                attn, ps, EXP, scale=SCALE,
                bias=mb_sb[:, kb:kb + 1] if use_mask else 0.0)
        return attn

    def ctx_pair(pc, kb, attn, first, last):
        nc.tensor.matmul(pc[0:64, :], v_all[:, kb, 0:64],
                         attn[:, 0:512], start=first, stop=last,
                         skip_group_check=True)
        nc.tensor.matmul(pc[64:128, :], v_all[:, kb, 64:128],
                         attn[:, 512:1024], start=first, stop=last,
                         skip_group_check=True)

    fifo = [scores_block(g) for g in range(4)]
    pc = pd = None
    for u in range(NBLK // 2):
        g0, g1 = 2 * u, 2 * u + 1
        qb, kb0 = divmod(g0, KB)
        kb1 = kb0 + 1
        qs = slice(qb * Q_BLK, (qb + 1) * Q_BLK)
        if kb0 == 0:
            pc = psum.tile([128, 512], F32, tag="ctx", bufs=2, name="pc")
            pd = psum.tile([97, 512], F32, tag="den", bufs=2, name="pd")
        for stage in pending.pop(u, ()):
            stage()

        a0 = fifo.pop(0)
        ctx_pair(pc, kb0, a0, kb0 == 0, False)
        if g0 + 4 < NBLK:
            fifo.append(scores_block(g0 + 4))
        a1 = fifo.pop(0)
        ctx_pair(pc, kb1, a1, False, kb1 == KB - 1)
        if g1 + 4 < NBLK:
            fifo.append(scores_block(g1 + 4))

        # fused denominator slot: 4 concurrent M=1 matmuls, one per col
        # group; even block sums land on rows 0/32, odd block on 64/96
        for (row, att, sl) in ((0, a0, slice(0, 512)), (32, a0, slice(512, 1024)),
                               (64, a1, slice(0, 512)), (96, a1, slice(512, 1024))):
            nc.tensor.matmul(pd[row:row + 1, :], ones_col[:, 0:1], att[:, sl],
                             start=(kb0 == 0), stop=(kb1 == KB - 1),
                             skip_group_check=True,
                             tile_position=(0, row))

        if kb1 != KB - 1:
            continue

        # ---- qb tail ----
        # pd -> sbuf f16 (clamped so psum garbage in unused rows can't
        # poison the selector matmul with inf), then dsum = sel.T @ pd_sb
        # sums the even/odd halves per head; reciprocal + f16 cast feed
        # the PE broadcast. All emitted eagerly so the den-ring slots are
        # freed in allocation order; muls/oproj staggered into qb+1.
        pd_sb = sp.tile([97, 512], F16, tag="pd_sb", bufs=2)
        nc.vector.tensor_scalar(pd_sb, pd, 60000.0, -60000.0,
                                mybir.AluOpType.min, mybir.AluOpType.max)
        prd = psum.tile([128, 512], F32, tag="den", bufs=2, name="prd")
        nc.tensor.matmul(prd, sel4, pd_sb)
        rep = sp.tile([128, 512], F32, tag="rep", bufs=2, name="rep")
        nc.vector.reciprocal_approx_fast(rep, prd)

        def muls(i, qb=qb, pc=pc, rep=rep):
            cs = slice(i * 128, (i + 1) * 128)
            qcs = slice(qb * Q_BLK + i * 128, qb * Q_BLK + (i + 1) * 128)
            nc.vector.tensor_mul(ctxn[:, qcs], pc[:, cs], rep[:, cs])

        def oproj(i, qb=qb):
            sb = qb * (Q_BLK // 128) + i
            po = psum.tile([128, 512], F32, tag="ctx", bufs=2, name="po")
            nc.tensor.matmul(po, ctxn[:, sb * 128:(sb + 1) * 128], wo_sb)
            ob = sp.tile([128, 512], F32, tag="ob", bufs=3, name="ob")
            nc.vector.tensor_copy(ob, po)
            nc.sync.dma_start(d["out"].ap()[sb * 128:(sb + 1) * 128, :], ob)

        if qb == QB - 1:
            # eager, pipelined per 128-query slice
            for i in range(4):
                muls(i)
                oproj(i)
        else:
            base = 16 * (qb + 1)
            for off, i in zip((2, 4, 6, 8), range(4)):
                pending.setdefault(base + off, []).append(
                    lambda i=i, m=muls, o=oproj: (m(i), o(i)))

    for u in sorted(pending):
        for stage in pending[u]:
            stage()


def build_program(S=4096, use_mask=False, use_bq=False, use_bk=False,
                  use_bv=False, enable_asserts=False):
    nc = bacc.Bacc("TRN2", target_bir_lowering=False, debug=False,
                   enable_asserts=enable_asserts, num_devices=N_CORES,
                   name="mha")
    d = {
        "xt": nc.dram_tensor("xt", [D_MODEL, S], BF16, kind="ExternalInput"),
        "wq": nc.dram_tensor("wq", [D_MODEL, DL], BF16, kind="ExternalInput"),
        "wk": nc.dram_tensor("wk", [D_MODEL, DL], BF16, kind="ExternalInput"),
        "wv": nc.dram_tensor("wv", [D_MODEL, DL], BF16, kind="ExternalInput"),
        "wo": nc.dram_tensor("wo", [DL, D_MODEL], BF16, kind="ExternalInput"),
        "sel4": nc.dram_tensor("sel4", [97, DL], F16, kind="ExternalInput"),
        "idn": nc.dram_tensor("idn", [DL, DL], BF16, kind="ExternalInput"),
        "out": nc.dram_tensor("out", [S, D_MODEL], F32, kind="ExternalOutput"),
    }
    if use_bq:
        d["bq"] = nc.dram_tensor("bq", [DL], F32, kind="ExternalInput")
    if use_bk:
        d["bk"] = nc.dram_tensor("bk", [DL], F32, kind="ExternalInput")
    if use_bv:
        d["bv"] = nc.dram_tensor("bv", [DL], F32, kind="ExternalInput")
    if use_mask:
        d["mb"] = nc.dram_tensor("mb", [128, S // 128], F32,
                                 kind="ExternalInput")
    with tile.TileContext(nc) as tc:
        with ExitStack() as ctx:
            build_kernel(ctx, tc, S, use_mask, use_bq, use_bk, use_bv, d)
    nc.compile()
    return nc


def _sel4():
    # [97, 128] f16: col c sums pd rows {0,64} (head0, c<64) or {32,96}
    # (head1, c>=64), broadcasting the summed denominator across cols
    s = np.zeros((97, DL), np.float16)
    s[0, 0:64] = s[64, 0:64] = 1.0
    s[32, 64:128] = s[96, 64:128] = 1.0
    return s


_cache = {}


def _program(key):
    if key not in _cache:
        _cache[key] = build_program(
            S=4096, use_mask=key[0], use_bq=key[1], use_bk=key[2],
            use_bv=key[3])
    return _cache[key]


def kernel(x, mask, Wq, bq, Wk, bk, Wv, bv, Wo, bo, _results_hook=None):
    x = np.asarray(x, np.float32)
    mask = np.asarray(mask)
    B, S, _ = x.shape
    use_mask = bool((mask == 0).any())
    use_bq = bool(np.asarray(bq).any())
    use_bk = bool(np.asarray(bk).any())
    use_bv = bool(np.asarray(bv).any())
    nc = _program((use_mask, use_bq, use_bk, use_bv))

    in_maps = []
    for c in range(N_CORES):
        b, j = divmod(c, N_CORES // B)
        ds = slice(j * DL, (j + 1) * DL)
        m = {
            "xt": np.ascontiguousarray(x[b].T).astype(ml_dtypes.bfloat16),
            "wq": np.ascontiguousarray(Wq[:, ds]).astype(ml_dtypes.bfloat16),
            "wk": np.ascontiguousarray(Wk[:, ds]).astype(ml_dtypes.bfloat16),
            "wv": np.ascontiguousarray(Wv[:, ds]).astype(ml_dtypes.bfloat16),
            "wo": np.ascontiguousarray(Wo[ds, :]).astype(ml_dtypes.bfloat16),
            "sel4": _sel4(),
            "idn": np.eye(DL, dtype=ml_dtypes.bfloat16),
        }
        if use_bq:
            m["bq"] = np.ascontiguousarray(bq[ds], dtype=np.float32)
        if use_bk:
            m["bk"] = np.ascontiguousarray(bk[ds], dtype=np.float32)
        if use_bv:
            m["bv"] = np.ascontiguousarray(bv[ds], dtype=np.float32)
        if use_mask:
            mb = np.where(np.asarray(mask[b]) == 0, -1e9, 0.0).astype(np.float32)
            m["mb"] = np.ascontiguousarray(mb.reshape(S // 128, 128).T)
        in_maps.append(m)

    res = run_bass_kernel_spmd(nc, in_maps, core_ids=list(range(N_CORES)))
    if _results_hook is not None:
        _results_hook(res)
    out = np.zeros((B, S, D_MODEL), np.float32)
    for c in range(N_CORES):
        b = c // (N_CORES // B)
        out[b] += res.results[c]["out"]
    out += np.asarray(bo, np.float32)
    return out
